# revision 21
# baseline (speedup 1.0000x reference)
"""Trainium2 Bass kernel for nn_MultiHeadAttnC (QANet-style self-attention).

Reference computation (per batch b):
    memory = w_mem @ queries[b]          # [2D, L]  (pointwise conv)
    query  = w_query @ queries[b]        # [D, L]
    K, V   = heads of memory             # H=8 heads, DH=16
    Q      = heads of query * DH^-0.5
    S      = Q @ K^T  (masked over kv)   # [H, L, L]
    out[b] = softmax(S) @ V  -> recombined to [D, L]

Strategy:
  - Data parallel: batch b -> NeuronCore b. Weights replicated. No collectives.
  - K-major ("transposed") attention per head: S^T[kv, q] = K^T.T @ Q^T with
    kv positions on PSUM partitions, computed with 4 heads concurrently via
    tensor-engine row tiling (contraction dim is only DH=16 -> 32-row groups).
  - Multiplicative 0/1 mask folds into a per-position validity vector that is
    multiplied into V and into an extra all-ones lhsT column, so exp(S) of a
    masked position contributes exactly 0 to both the numerator and the
    softmax denominator (exp never needs a mask bias, result is exact).
  - exp runs on the scalar engine straight out of PSUM (4 banks = N=2048 per
    instruction) into SBUF; the A/B head-group PSUM regions alternate so ACT
    stays 100% busy (it is the roofline engine: B*H*L^2 = 268M exps chip-wide).
  - AV matmul: out^T[dh, q] = [V | valid].T @ P^T via 32-column tiling (M=17:
    16 output channels + softmax denominator), accumulated across kv chunks
    on the vector engine in SBUF.
  - Final normalize: reciprocal of the denominator rows, DMA partition
    permute/broadcast to the output layout, one multiply. Output tile is
    already [D, L] — matches the reference's final transpose for free.
"""

import numpy as np
from contextlib import ExitStack

import concourse.bass as bass
import concourse.tile as tile
from concourse import bacc, mybir
from concourse import bass_utils

B, D, L, H, DH = 8, 128, 2048, 8, 16
f32 = mybir.dt.float32
bf16 = mybir.dt.bfloat16
f32r = mybir.dt.float32r
S_DT = f32r    # dtype of K/Q spread tiles (S^T matmul inputs; row tiling+f32r ok)
AV_DT = bf16   # dtype of V tiles and exp output P (col tiling rejects f32r)
IN_DT = f32r   # dtype tag of DRAM inputs / projection matmul inputs
QTILE = 512          # q columns per inner tile (one PSUM bank)
NJQ = L // QTILE     # 4

_program_cache: dict[int, "bacc.Bacc"] = {}


def _bcast_rows(t: bass.AP, f: int) -> bass.AP:
    """AP over tile t (free extent f) that reads partition rows
    {base, base+32, base+64, base+96}, each repeated 16x (DMA source only).
    Verified: dim0 steps flatly across partitions (pitch=f); the step-0 mid
    dim replicates; the DMA streams groups in order into a contiguous dest."""
    assert t.shape[0] == 1
    return bass.AP(tensor=t.tensor, offset=t.offset,
                   ap=[[32 * f, 4], [0, 16], [1, f]])


def _body(ctx, tc, qf_d, qkv_d, wq_d, wk_d, wv_d, val_d, out_d, n_kv):
    nc = tc.nc
    Lkv = n_kv * 128
    Exp = mybir.ActivationFunctionType.Exp

    consts = ctx.enter_context(tc.tile_pool(name="consts", bufs=1))

    qf = consts.tile([D, L], IN_DT, tag="qf")
    for j in range(NJQ):
        nc.sync.dma_start(out=qf[:, j * QTILE:(j + 1) * QTILE],
                          in_=qf_d[:, j * QTILE:(j + 1) * QTILE])
    qkv = consts.tile([D, Lkv], IN_DT, tag="qkv")
    col = 0
    while col < Lkv:
        n = min(512, Lkv - col)
        nc.sync.dma_start(out=qkv[:, col:col + n], in_=qkv_d[:, col:col + n])
        col += n
    wq = consts.tile([D, 2, D], IN_DT, tag="wq")
    wk = consts.tile([D, 2, D], IN_DT, tag="wk")
    for X in range(2):
        nc.sync.dma_start(out=wq[:, X, :], in_=wq_d[X])
        nc.sync.dma_start(out=wk[:, X, :], in_=wk_d[X])
    wv = consts.tile([D, D], IN_DT, tag="wv")
    nc.sync.dma_start(out=wv, in_=wv_d)
    val = consts.tile([128, n_kv], f32, tag="val")
    nc.sync.dma_start(out=val, in_=val_d)
    ones8 = consts.tile([128, 8], f32, tag="ones8")
    nc.vector.memset(ones8, 1.0)

    q_sp = consts.tile([D, 2, L], S_DT, tag="q_sp")     # Q^T spread (pre-scaled)
    k_sp = consts.tile([D, 2, Lkv], S_DT, tag="k_sp")   # K^T spread
    v_sb = consts.tile([128, n_kv, H, DH + 1], AV_DT, tag="v_sb")
    out_sb = consts.tile([D, L], f32, tag="out_sb")

    # Two 4-bank PSUM regions (A/B head groups); all PSUM flows through them.
    spools = [
        ctx.enter_context(tc.tile_pool(name="spsA", bufs=1, space="PSUM")),
        ctx.enter_context(tc.tile_pool(name="spsB", bufs=1, space="PSUM")),
    ]

    def s_tile(X):
        return spools[X].tile([128, 4 * QTILE], f32, tag=f"s{X}", name=f"s{X}")

    # ---- projections ----
    for X in range(2):
        ps = s_tile(X)
        for j in range(NJQ):
            nc.tensor.matmul(ps[:, j * QTILE:(j + 1) * QTILE], lhsT=wq[:, X, :],
                             rhs=qf[:, j * QTILE:(j + 1) * QTILE],
                             start=True, stop=True)
            nc.vector.tensor_copy(out=q_sp[:, X, j * QTILE:(j + 1) * QTILE],
                                  in_=ps[:, j * QTILE:(j + 1) * QTILE])
        ps2 = s_tile(X)
        col = 0
        while col < Lkv:
            n = min(512, Lkv - col)
            nc.tensor.matmul(ps2[:, col:col + n], lhsT=wk[:, X, :],
                             rhs=qkv[:, col:col + n], start=True, stop=True)
            nc.vector.tensor_copy(out=k_sp[:, X, col:col + n],
                                  in_=ps2[:, col:col + n])
            col += n
    for c in range(n_kv):
        vp = s_tile(c % 2)
        nc.tensor.matmul(vp[:, 0:D], lhsT=qkv[:, c * 128:(c + 1) * 128],
                         rhs=wv, start=True, stop=True)
        # V columns scaled by validity (zeroes masked kv positions exactly)
        nc.vector.tensor_scalar_mul(
            v_sb[:, c, :, 0:DH],
            vp[:, 0:D].rearrange("p (h x) -> p h x", x=DH),
            val[:, c:c + 1])
        # ones column * validity -> softmax denominator source
        nc.vector.tensor_scalar_mul(
            v_sb[:, c, :, DH:DH + 1],
            ones8.rearrange("p (h x) -> p h x", x=1),
            val[:, c:c + 1])

    # ---- attention ----
    p_pool = ctx.enter_context(tc.tile_pool(name="p_pool", bufs=4))
    acc_pool = ctx.enter_context(tc.tile_pool(name="acc", bufs=2))
    misc = ctx.enter_context(tc.tile_pool(name="misc", bufs=2))

    # Four interleaved streams (2 q-chunks x 2 head-groups) over the two
    # 4-bank PSUM regions: each region is revisited only every other exp
    # slot, so its AV quad + DVE drain + next S quad all hide under the
    # other streams' exps and ACT runs back-to-back.
    for jp in range(NJQ // 2):
        streams = [(2 * jp, 0), (2 * jp, 1), (2 * jp + 1, 0), (2 * jp + 1, 1)]
        accs = {s: acc_pool.tile([128, QTILE], f32, tag=f"acc{i}",
                                 name=f"acc{i}")
                for i, s in enumerate(streams)}
        pend = [None, None]  # per PSUM region: (sp, p_sb, c, stream)

        def flush_av(R):
            sp, p_sb, c, s = pend[R]
            X = s[1]
            # AV quad into (consumed) bank 0 of that iteration's PSUM region:
            # out^T[dh,q] += [V|valid].T @ P^T  (col tiling, M=17)
            for g in range(4):
                h = X * 4 + g
                nc.tensor.matmul(
                    sp[g * 32:g * 32 + DH + 1, 0:QTILE],
                    lhsT=v_sb[:, c, h, :],
                    rhs=p_sb[:, g * QTILE:(g + 1) * QTILE],
                    start=True, stop=True, tile_position=(0, g * 32))
            if c == 0:
                nc.vector.tensor_copy(out=accs[s], in_=sp[:, 0:QTILE])
            else:
                nc.vector.tensor_add(out=accs[s], in0=accs[s],
                                     in1=sp[:, 0:QTILE])
            pend[R] = None

        for c in range(n_kv):
            for jq, X in streams:
                if pend[X] is not None:
                    flush_av(X)
                qs = slice(jq * QTILE, (jq + 1) * QTILE)
                sp = s_tile(X)
                # S^T quad: 4 heads concurrently (row tiling, K=32 incl. 0s)
                for g in range(4):
                    nc.tensor.matmul(
                        sp[:, g * QTILE:(g + 1) * QTILE],
                        lhsT=k_sp[g * 32:(g + 1) * 32, X, c * 128:(c + 1) * 128],
                        rhs=q_sp[g * 32:(g + 1) * 32, X, qs],
                        start=True, stop=True, tile_position=(g * 32, 0))
                p_sb = p_pool.tile([128, 4 * QTILE], AV_DT, tag="p")
                nc.scalar.activation(out=p_sb, in_=sp, func=Exp)
                pend[X] = (sp, p_sb, c, (jq, X))
        for R in range(2):
            flush_av(R)

        # ---- per-pair epilogue: normalize + assemble output layout ----
        for jq in (2 * jp, 2 * jp + 1):
            qs = slice(jq * QTILE, (jq + 1) * QTILE)
            recs = [misc.tile([128, QTILE], f32, tag=f"rec{X}", name=f"rec{X}")
                    for X in range(2)]
            for X in range(2):
                nc.vector.reciprocal(out=recs[X], in_=accs[(jq, X)])
            xt = misc.tile([128, QTILE], f32, tag="xt")
            rb = misc.tile([128, QTILE], f32, tag="rb")
            for X in range(2):
                for g in range(4):
                    h = X * 4 + g
                    nc.sync.dma_start(out=xt[h * DH:(h + 1) * DH, :],
                                      in_=accs[(jq, X)][g * 32:g * 32 + DH, :])
                nc.sync.dma_start(
                    out=rb[X * 64:(X + 1) * 64, :],
                    in_=_bcast_rows(recs[X][DH:DH + 1, :], QTILE))
            nc.vector.tensor_mul(out=out_sb[:, qs], in0=xt, in1=rb)
            nc.sync.dma_start(out=out_d[:, qs], in_=out_sb[:, qs])


def _build(n_kv: int) -> "bacc.Bacc":
    Lkv = n_kv * 128
    nc = bacc.Bacc("TRN2", target_bir_lowering=False, debug=False,
                   enable_asserts=True, num_devices=B)
    qf_d = nc.dram_tensor("q_full", [D, L], IN_DT, kind="ExternalInput").ap()
    qkv_d = nc.dram_tensor("q_kv", [D, Lkv], IN_DT, kind="ExternalInput").ap()
    wq_d = nc.dram_tensor("wq_sp", [2, D, D], IN_DT, kind="ExternalInput").ap()
    wk_d = nc.dram_tensor("wk_sp", [2, D, D], IN_DT, kind="ExternalInput").ap()
    wv_d = nc.dram_tensor("wv_t", [D, D], IN_DT, kind="ExternalInput").ap()
    val_d = nc.dram_tensor("valid", [128, n_kv], f32, kind="ExternalInput").ap()
    out_d = nc.dram_tensor("out", [D, L], f32, kind="ExternalOutput").ap()

    with tile.TileContext(nc) as tc, ExitStack() as ctx:
        _body(ctx, tc, qf_d, qkv_d, wq_d, wk_d, wv_d, val_d, out_d, n_kv)
    nc.compile()
    return nc


def _prep_weights(w_mem: np.ndarray, w_query: np.ndarray):
    """Spread head weights into 32-row tile groups (rows 16:32 zero) and
    pre-transpose for use as matmul lhsT. Q gets the DH^-0.5 scale."""
    wq_sp = np.zeros((2, D, D), np.float32)
    wk_sp = np.zeros((2, D, D), np.float32)
    scale = np.float32(DH ** -0.5)
    for X in range(2):
        for g in range(4):
            h = 4 * X + g
            wq_sp[X][:, 32 * g:32 * g + DH] = (w_query[DH * h:DH * (h + 1), :] * scale).T
            wk_sp[X][:, 32 * g:32 * g + DH] = w_mem[DH * h:DH * (h + 1), :].T
    wv_t = np.ascontiguousarray(w_mem[D:2 * D, :].T)
    return wq_sp, wk_sp, wv_t


COMPACT_KV = True  # drop masked kv positions host-side (exact: they contribute
                   # exp(-1e30)=0 to softmax numerator and denominator alike)


def prepare(queries: np.ndarray, mask: np.ndarray, w_mem: np.ndarray,
            w_query: np.ndarray):
    """Build (compiled program, per-core input maps)."""
    assert queries.shape == (B, D, L) and mask.shape == (B, L)
    maskf = mask.astype(np.float32)
    kept = [np.nonzero(maskf[b] > 0.0)[0] for b in range(B)]
    if COMPACT_KV and all(len(k) > 0 for k in kept):
        n_kv = max(1, -(-max(len(k) for k in kept) // 128))
    else:
        n_kv = L // 128
        kept = None
    Lkv = n_kv * 128

    nc = _program_cache.get(n_kv)
    if nc is None:
        nc = _program_cache[n_kv] = _build(n_kv)

    wq_sp, wk_sp, wv_t = _prep_weights(
        w_mem.astype(np.float32), w_query.astype(np.float32))

    in_maps = []
    for b in range(B):
        qb = np.ascontiguousarray(queries[b], dtype=np.float32)
        if kept is not None:
            idx = kept[b]
            qkv = np.zeros((D, Lkv), np.float32)
            qkv[:, :len(idx)] = qb[:, idx]
            val = np.zeros(Lkv, np.float32)
            val[:len(idx)] = 1.0
        else:
            qkv = qb
            val = maskf[b]
        in_maps.append({
            "q_full": qb,
            "q_kv": np.ascontiguousarray(qkv),
            "wq_sp": wq_sp,
            "wk_sp": wk_sp,
            "wv_t": wv_t,
            "valid": np.ascontiguousarray(val.reshape(n_kv, 128).T),
        })
    return nc, in_maps


def kernel(queries: np.ndarray, mask: np.ndarray, w_mem: np.ndarray,
           w_query: np.ndarray) -> np.ndarray:
    nc, in_maps = prepare(queries, mask, w_mem, w_query)
    res = bass_utils.run_bass_kernel_spmd(nc, in_maps, core_ids=list(range(B)))
    return np.stack([res.results[b]["out"] for b in range(B)]).astype(np.float32)
